# revision 1
# baseline (speedup 1.0000x reference)
"""QAM64 constellation unmapper (nearest-neighbor argmin) on 8 TRN2 cores.

The reference computes argmin_m ||x[:, n] - c[:, m]|| over an 8x8 QAM grid
c = levels x levels / sqrt(42), levels = {-7,-5,...,7}.  For a uniform grid
the nearest-neighbor index factorizes per coordinate:

    qI = clip(round(xI * a + 3.5), 0, 7),  a = sqrt(42)/2
    qQ = clip(round(xQ * a + 3.5), 0, 7)
    idx = 8*qI + qQ

(round = RNE; verified bit-exact against the jax reference for the fixed
problem input on both the CPU and neuron backends.)

Device kernel (fused8 variant; per core, data-parallel over N), all in f32
with magic-number RNE rounding so no dependence on convert rounding modes.
Both rows share ONE affine (computing 8*q per row), so each chunk needs a
single ScalarE activation over [128, 2F]:

    v8 = Relu(x * 8a + 28)         # ScalarE, both rows: 8*(a*x+3.5), >= 0
    u  = min(v8, 59.5) + M8        # DVE ts: upper clamp + magic RNE to the
                                   #   spacing-8 grid (M8 = 1.5*2^26)
    qQ = u[Q] * 0.125 - M1         # DVE ts: exact (power-of-2 scale;
                                   #   M8*0.125 = M1 = 1.5*2^23)
    out = int8((u[I] - M8) + qQ)   # DVE scalar_tensor_tensor, exact 0..63

GpSimd is deliberately unused: its f32 tensor_scalar measures ~3.8 us per
[128,256] op on HW (~8x the cost model) and stalls concurrent DVE work via
SBUF port sharing.  Output is stored as int8 on device and widened to int32
on the host after the gather.
"""

import numpy as np

import concourse.bass as bass
import concourse.tile as tile
from concourse import mybir
from concourse.bass_utils import run_bass_kernel_spmd

N_TOTAL = 1_048_576
N_CORES = 8
N_C = N_TOTAL // N_CORES  # 131072 symbols per core
P = 128
F_TOTAL = N_C // P  # 1024 symbols per partition
CHUNKS = (256, 256, 256, 192, 64)

A = np.float32(np.sqrt(np.float32(42.0)) / 2.0)  # = s/2, exact in f32
A8 = np.float32(8.0 * A)
M1 = float(np.float32(1.5 * 2.0**23))
M8 = float(np.float32(1.5 * 2.0**26))
M81 = float(np.float32(M8 + M1))  # exactly representable

_cache = {}


def _strip_preamble(nc):
    """Drop the const-AP memsets and the init all-engine barrier: this kernel
    never reads the built-in const APs, so they are dead code (~1us)."""
    bb = nc.m.functions[0].blocks[0]
    dead = ("InstMemset", "InstDrain", "InstEventSemaphore")
    bb.instructions = [i for i in bb.instructions if type(i).__name__ not in dead]


def _split_waits(nc, cap=1):
    """Walrus caps sync waits per instruction (~8 for CTRL, 1 for compute).
    Tile's final drain aggregates one wait per DMA-sem lane plus engine sems
    and can exceed the cap; peel excess waits onto no-op carriers in front."""
    for bb in nc.m.functions[0].blocks:
        insts = list(bb.instructions)
        out = []
        changed = False
        for i in insts:
            si = i.sync_info
            w = list(si.on_wait) if (si is not None and si.on_wait) else []
            if len(w) > cap:
                extra, keep = w[:-cap], w[-cap:]
                k = 0
                while extra:
                    grp, extra = extra[:cap], extra[cap:]
                    nop = mybir.InstNoOp(
                        name=f"{i.name}-presync{k}", engine=i.engine
                    )
                    nop.sync_info = mybir.SyncInfo(on_wait=grp, on_update=[])
                    out.append(nop)
                    k += 1
                i.sync_info = mybir.SyncInfo(
                    on_wait=keep, on_update=list(si.on_update)
                )
                changed = True
            out.append(i)
        if changed:
            bb.instructions = out


def _strip_epilogue(nc):
    """Drop Tile's end-of-kernel barrier butterfly (EventSemaphore rounds):
    each engine just drains its own work and halts; the NRT-level end
    barrier outside the kernel span handles process completion."""
    for bb in nc.m.functions[0].blocks:
        if not bb.name.endswith("_end"):
            continue
        bb.instructions = [
            i for i in bb.instructions if type(i).__name__ != "InstEventSemaphore"
        ]


def _build(
    chunks=CHUNKS,
    combine="fused8",
    strip=True,
    reps=1,
    groups=None,
    ts2_act=2,
    epi_strip=True,
):
    assert sum(chunks) == F_TOTAL
    if groups is None:
        groups = (len(chunks),)  # one store per group of chunks
    assert sum(groups) == len(chunks)
    nc = bass.Bass(
        "TRN2", target_bir_lowering=False, debug=False, num_devices=N_CORES
    )
    if strip:
        _strip_preamble(nc)

    x_d = nc.dram_tensor("x", [2, N_C], mybir.dt.float32, kind="ExternalInput")
    o_d = nc.dram_tensor("out", [N_C], mybir.dt.int8, kind="ExternalOutput")

    # [128, 2, 1024]: partition-major view of each row; one DMA loads I+Q
    x3 = x_d.ap().rearrange("r (p f) -> p r f", p=P)
    out = o_d.ap().rearrange("(p f) -> p f", p=P)

    f32 = mybir.dt.float32
    Relu = mybir.ActivationFunctionType.Relu
    Copy = mybir.ActivationFunctionType.Copy
    Op = mybir.AluOpType

    nch = len(chunks)
    with tile.TileContext(nc) as tc:
        with (
            tc.tile_pool(name="cst", bufs=1) as cst_pool,
            tc.tile_pool(name="io", bufs=nch) as io_pool,
            tc.tile_pool(name="tmp", bufs=nch) as tmp_pool,
            tc.tile_pool(name="ot", bufs=nch) as out_pool,
        ):
            b28 = cst_pool.tile([P, 1], f32, tag="b28")
            nc.vector.memset(b28[:], 28.0)
            b35 = cst_pool.tile([P, 1], f32, tag="b35")
            nc.vector.memset(b35[:], 3.5)
            scr = cst_pool.tile([P, 1], f32, tag="scr")
            # ScalarE warmup: reads both bias tiles so the DVE-memset wait
            # lands here once; the ISA allows only one sync wait per compute
            # instruction, and the real activations need theirs for the
            # input-load semaphore.
            scrA = cst_pool.tile([P, 1], f32, tag="scrA")
            nc.scalar.activation(scrA[:], b35[:], Relu, bias=b28[:], scale=1.0)

            for _ in range(reps):
                # Issue all loads up front; SP sequencer streams them.
                loads = []
                off = 0
                for F in chunks:
                    t = io_pool.tile([P, 2, F], f32, tag=f"in{off}")
                    nc.sync.dma_start(t[:], x3[:, :, off : off + F])
                    loads.append((t, off, F))
                    off += F

                # Group chunks per store: one shared int8 tile per group so a
                # single DMA stores the whole group (stt's write in-order on
                # DVE; the store carries exactly one wait).
                gi = iter(loads)
                ci = -1
                for gsz in groups:
                    grp = [next(gi) for _ in range(gsz)]
                    g_off = grp[0][1]
                    g_len = sum(F for _, _, F in grp)
                    ot = out_pool.tile([P, g_len], mybir.dt.int8, tag=f"ot{g_off}")
                    for t, off, F in grp:
                        ci += 1
                        sl = slice(off - g_off, off - g_off + F)
                        last = off + F == F_TOTAL
                        ts2_on_act = ci < ts2_act
                        if combine == "fused8":
                            # Same affine for BOTH rows: v8 = Relu(8a*x + 28)
                            # computes 8*q on each row in ONE activation.
                            # No GpSimd anywhere: its f32 tensor_scalar runs
                            # ~3.8us per [128,256] op on HW and stalls
                            # concurrent DVE work via SBUF port sharing.
                            v8 = tmp_pool.tile([P, 2, F], f32, tag="v8")
                            nc.scalar.activation(
                                v8[:, :, :], t[:, :, :], Relu,
                                bias=b28[:], scale=float(A8),
                            )
                            u = tmp_pool.tile([P, 2, F], f32, tag="u")
                            nc.vector.tensor_scalar(
                                u[:, :, :], v8[:, :, :], 59.5, M8,
                                op0=Op.min, op1=Op.add,
                            )
                            # Q: qQ = u*0.125 - M1; both steps exact in f32.
                            qQt = tmp_pool.tile([P, F], f32, tag="qQt")
                            if last or not ts2_on_act:
                                nc.vector.tensor_scalar(
                                    qQt[:], u[:, 1, :], 0.125, M1,
                                    op0=Op.mult, op1=Op.subtract,
                                )
                            else:
                                # ScalarE Copy(scale*in + bias) with float
                                # bias: offloads the descale from DVE.
                                nc.scalar.activation(
                                    qQt[:], u[:, 1, :], Copy,
                                    bias=-M1, scale=0.125,
                                )
                                # Wait-carrier (ACT -> DVE) for the STT.
                                nc.vector.tensor_copy(scr[:], qQt[:, 0:1])
                            # out = (uI - M8) + qQ = 8*qI + qQ, exact
                            nc.vector.scalar_tensor_tensor(
                                ot[:, sl], u[:, 0, :], M8, qQt[:],
                                op0=Op.subtract, op1=Op.add,
                            )
                            continue
                        # Q chain first: it goes through the slower Pool engine.
                        # The final chunk keeps its Q path on DVE: no
                        # cross-engine hop in the tail-latency chain.
                        vQ = tmp_pool.tile([P, F], f32, tag="vQ")
                        nc.scalar.activation(
                            vQ[:], t[:, 1, :], Relu, bias=b35[:], scale=float(A)
                        )
                        uQ = tmp_pool.tile([P, F], f32, tag="uQ")
                        q_eng = nc.vector if last else nc.gpsimd
                        q_eng.tensor_scalar(
                            uQ[:], vQ[:], 7.4375, M1, op0=Op.min, op1=Op.add
                        )

                        vI = tmp_pool.tile([P, F], f32, tag="vI")
                        nc.scalar.activation(
                            vI[:], t[:, 0, :], Relu, bias=b28[:], scale=float(A8)
                        )
                        uI = tmp_pool.tile([P, F], f32, tag="uI")
                        nc.vector.tensor_scalar(
                            uI[:], vI[:], 59.5, M8, op0=Op.min, op1=Op.add
                        )

                        if combine == "stt":
                            if not last:
                                # Wait-carrier: pulls the Pool->DVE semaphore
                                # wait onto a cheap op so the STT (one wait
                                # slot in the ISA struct) needs none.
                                nc.vector.tensor_copy(scr[:], uQ[:, 0:1])
                            # out = (uI - (M8+M1)) + uQ = 8*qI + qQ, exact
                            nc.vector.scalar_tensor_tensor(
                                ot[:, sl], uI[:], M81, uQ[:],
                                op0=Op.subtract, op1=Op.add,
                            )
                        else:
                            wI = tmp_pool.tile([P, F], f32, tag="wI")
                            nc.vector.tensor_scalar(
                                wI[:], uI[:], M81, None, op0=Op.subtract
                            )
                            nc.vector.tensor_tensor(ot[:, sl], wI[:], uQ[:], op=Op.add)
                    nc.sync.dma_start(out[:, g_off : g_off + g_len], ot[:])
    if epi_strip:
        _strip_epilogue(nc)
    _split_waits(nc)
    return nc


def kernel(x: np.ndarray, constellation: np.ndarray, **run_kwargs) -> np.ndarray:
    if "nc" not in _cache:
        _cache["nc"] = _build(
            chunks=(256, 256, 256, 256), groups=(3, 1), combine="fused8", ts2_act=0, epi_strip=False
        )
    nc = _cache["nc"]

    xs = np.asarray(x, dtype=np.float32).reshape(2, N_TOTAL)
    in_maps = [
        {"x": np.ascontiguousarray(xs[:, c * N_C : (c + 1) * N_C])}
        for c in range(N_CORES)
    ]
    res = run_bass_kernel_spmd(nc, in_maps, core_ids=list(range(N_CORES)), **run_kwargs)
    out = np.concatenate([r["out"].reshape(-1) for r in res.results])
    result = out.astype(np.int32).reshape(1, 1, 1, N_TOTAL)
    _cache["last_results"] = res
    return result

